# revision 23
# baseline (speedup 1.0000x reference)
"""LDW-upsample (lifting wavelet) kernel for 8 Trainium2 NeuronCores.

The reference module is linear in x:
    out[b, j, 2h+r, 2w+s] = sum_c Weff_{r,s}[j, c] * x[b, c, h, w]
where Weff folds the 1x1-conv weight and the 4 lifting filter taps, so the
whole module is one 256->256 1x1 conv + a 2x2 pixel-shuffle.

Sharding: pure data parallel, 2 batch images per core.

Per-core dataflow (raw bass, 4 engines, manual semaphores):
  - PSUM/SBUF output partitions carry bj = b*64 + j (both images at once) so
    every DMA AP is <=3 dims at full 128 partitions (bj has uniform DRAM
    stride). The two images are computed by column-tiled M=64 matmuls that
    run concurrently in the PE array.
  - SP: input DMAs (4 MiB blocks of 16 input rows, both images+k-tiles);
    block 0 lands as 4 per-superchunk sub-DMAs so the first matmul starts
    after ~1/4 block (ramp).
  - PE: fp32 matmuls, k accumulated in PSUM; one PSUM bank per (r, s),
    double buffered (8 banks exactly).
  - ACT evicts s=0 PSUM, DVE evicts s=1 PSUM, stride-2 free-dim writes into
    a shared SBUF tile holding fully interleaved output rows in DRAM order
    (r interleaved at row level); ACT issues a 1 MiB output DMA per evicted
    superchunk, 8 KiB fully-contiguous runs per partition on both sides
    (fine-grained drain -> short tail).
"""

import sys

for _p in ("/opt/trn_rl_repo",):
    if _p not in sys.path:
        sys.path.insert(0, _p)

import numpy as np

B, C, H, W = 16, 256, 128, 128
C4 = C // 4
N_CORES = 8
B_PER_CORE = B // N_CORES  # 2
H2, W2 = 2 * H, 2 * W

POS_PER_IMG = H * W  # 16384
BLK_POS = 2048  # input pixels per block (16 input rows), per image
BLK_ROWS = BLK_POS // W  # 16
N_BLK = POS_PER_IMG // BLK_POS  # 8 (each block covers BOTH images)
SC_POS = 512  # super-chunk pixels (4 input rows): one PSUM bank per (r,s)
SC_PER_BLK = BLK_POS // SC_POS  # 4
N_SC = N_BLK * SC_PER_BLK  # 32

_CACHE = {}


def _effective_weights(conv1x1_w, lp_v, hp_v, lp_h, hp_h):
    """Fold lifting taps into the conv weight.

    Returns w_all float32 [128, 512]: eight lhsT tiles side by side, tile
    index (r*2+s)*2+k each [c_in_ktile(128 part), j(64)], computed in f64.
    """
    Wd = conv1x1_w.astype(np.float64)
    lv = lp_v.reshape(C4, 2).astype(np.float64)
    hv = hp_v.reshape(C4, 2).astype(np.float64)
    lh = lp_h.reshape(C4, 2).astype(np.float64)
    hh = hp_h.reshape(C4, 2).astype(np.float64)

    va = np.stack([lv[:, 0], hv[:, 0]], axis=1)  # [j, r]
    vb = np.stack([lv[:, 1], hv[:, 1]], axis=1)
    hc0 = np.stack([lh[:, 0], hh[:, 0]], axis=1)  # [j, s]
    hc1 = np.stack([lh[:, 1], hh[:, 1]], axis=1)

    Wa, Wb, Wc, Wdq = Wd[:C4], Wd[C4 : 2 * C4], Wd[2 * C4 : 3 * C4], Wd[3 * C4 :]

    tiles = []
    for r in (0, 1):
        for s in (0, 1):
            Weff = (
                (hc0[:, s] * va[:, r])[:, None] * Wa
                + (hc0[:, s] * vb[:, r])[:, None] * Wb
                + (hc1[:, s] * va[:, r])[:, None] * Wc
                + (hc1[:, s] * vb[:, r])[:, None] * Wdq
            )  # [j, c]
            for k in (0, 1):
                tiles.append(Weff[:, k * 128 : (k + 1) * 128].T)  # [c, j]
    w_all = np.concatenate(tiles, axis=1)  # [128, 512]
    return np.ascontiguousarray(w_all.astype(np.float32))


def _build_nc(reps=1):
    """reps>1 repeats the whole pipeline (same data) inside one NEFF --
    benchmarking only, to scale the HW-exec signal above dispatch noise."""
    import concourse.bass as bass
    import concourse.mybir as mybir

    f32 = mybir.dt.float32
    nc = bass.Bass()

    xs = nc.declare_dram_parameter("xs", [B_PER_CORE, C, H, W], f32, isOutput=False)
    wp = nc.declare_dram_parameter("w", [128, 512], f32, isOutput=False)
    ys = nc.declare_dram_parameter("ys", [B_PER_CORE, C4, H2, W2], f32, isOutput=True)

    # Input view per block q: partition p = channel-within-ktile; free dims
    # (bk = b*2+k merged by uniform stride, pos contiguous).
    xv = xs[:].rearrange("b (k p) (q hh) w -> q p (b k) (hh w)", k=2, hh=BLK_ROWS)
    # Output view per block q: partition bj (uniform stride 65536), free =
    # the block's 32 output rows fully contiguous in DRAM (32*256 floats):
    # SBUF holds rows r-interleaved so one DMA per block moves 32 KiB
    # contiguous per partition.
    # Per-superchunk output view: superchunk sc covers output rows
    # sc*8 .. sc*8+8, fully contiguous per (b, j): 8 KiB runs.
    yvsc = ys[:].rearrange("b j (q zz) x -> q (b j) (zz x)", zz=2 * SC_POS // W)

    SLOT = 4 * BLK_POS  # 8192 floats per in/out slot

    from contextlib import ExitStack

    with ExitStack() as _stack:
        _e = _stack.enter_context
        w_all = _e(nc.sbuf_tensor("w_all", [128, 512], f32))
        in_buf = _e(nc.sbuf_tensor("in_buf", [128, 3 * SLOT], f32))
        out_buf = _e(nc.sbuf_tensor("out_buf", [128, 2 * SLOT], f32))
        ps000 = _e(nc.psum_tensor("ps000", [128, SC_POS], f32))
        ps010 = _e(nc.psum_tensor("ps010", [128, SC_POS], f32))
        ps100 = _e(nc.psum_tensor("ps100", [128, SC_POS], f32))
        ps110 = _e(nc.psum_tensor("ps110", [128, SC_POS], f32))
        ps001 = _e(nc.psum_tensor("ps001", [128, SC_POS], f32))
        ps011 = _e(nc.psum_tensor("ps011", [128, SC_POS], f32))
        ps101 = _e(nc.psum_tensor("ps101", [128, SC_POS], f32))
        ps111 = _e(nc.psum_tensor("ps111", [128, SC_POS], f32))
        w_sem = _e(nc.semaphore("w_sem"))
        in00_sem = _e(nc.semaphore("in00_sem"))
        in01_sem = _e(nc.semaphore("in01_sem"))
        in02_sem = _e(nc.semaphore("in02_sem"))
        in03_sem = _e(nc.semaphore("in03_sem"))
        inA_sem = _e(nc.semaphore("inA_sem"))
        inB_sem = _e(nc.semaphore("inB_sem"))
        inC_sem = _e(nc.semaphore("inC_sem"))
        mmA_sem = _e(nc.semaphore("mmA_sem"))
        mmV_sem = _e(nc.semaphore("mmV_sem"))
        evA_sem = _e(nc.semaphore("evA_sem"))
        evV_sem = _e(nc.semaphore("evV_sem"))
        od0_sem = _e(nc.semaphore("od0_sem"))
        od1_sem = _e(nc.semaphore("od1_sem"))
        block = _e(nc.Block())
        # ps[r][s][slot]
        ps = [[[ps000, ps001], [ps010, ps011]], [[ps100, ps101], [ps110, ps111]]]

        def wtile(r, s, k):
            i = (r * 2 + s) * 2 + k
            return w_all[:, i * 64 : (i + 1) * 64]

        def rhs(t, b, k, off, n):
            base = (t % 3) * SLOT + (b * 2 + k) * BLK_POS + off
            return in_buf[:, base : base + n]

        # out_buf as [p, rowpair(32), r(2), w(128), s(2)]: DRAM row order —
        # rowpair = slot*16 + hh (hh = input row within block), r interleaved
        # at row level so the whole slot is DRAM-contiguous per partition.
        obv = out_buf[:].rearrange("p (rp r w s) -> p rp r w s", r=2, w=128, s=2)

        NB = N_BLK * reps
        NSC = N_SC * reps

        # DMA-completion semaphores are per-consumer-chain: with several DMAs
        # on one ring incrementing a SHARED sem, per-engine skew lets
        # wait_ge(sem, 16*k) fire from mixed increments before DMA k fully
        # landed (observed: NaN on first exec / stale reads after). So:
        # block 0's sub-DMAs each get their own sem; later input blocks
        # rotate 3 sems by in_buf slot (same-sem users are 3 DMAs apart);
        # output DMAs rotate 2 sems by out_buf slot.
        in0_sems = [in00_sem, in01_sem, in02_sem, in03_sem]
        in_slot_sems = [inA_sem, inB_sem, inC_sem]
        od_sems = [od0_sem, od1_sem]

        @block.sync
        def _(sync: "bass.BassEngine"):
            sync.dma_start(out=w_all[:], in_=wp[:]).then_inc(w_sem, 16)
            for t in range(NB):
                if t >= 3:
                    # in_buf slot reuse: PE finished reading block t-3
                    sync.wait_ge(mmV_sem, SC_PER_BLK * (t - 2))
                if t == 0:
                    for cc in range(SC_PER_BLK):
                        sync.dma_start(
                            out=in_buf[:, :SLOT].rearrange(
                                "p (bk pos) -> p bk pos", bk=4
                            )[:, :, cc * SC_POS : (cc + 1) * SC_POS],
                            in_=xv[0][:, :, cc * SC_POS : (cc + 1) * SC_POS],
                        ).then_inc(in0_sems[cc], 16)
                else:
                    sync.dma_start(
                        out=in_buf[:, (t % 3) * SLOT : (t % 3) * SLOT + SLOT],
                        in_=xv[t % N_BLK],
                    ).then_inc(in_slot_sems[t % 3], 16)
            # all output DMAs done: blocks t%2==0 inc od0, t%2==1 inc od1
            sync.wait_ge(od0_sem, 16 * SC_PER_BLK * ((NB + 1) // 2))
            sync.wait_ge(od1_sem, 16 * SC_PER_BLK * (NB // 2))

        @block.tensor
        def _(tensor: "bass.BassEngine"):
            tensor.wait_ge(w_sem, 16)
            for sc in range(NSC):
                t, cc = divmod(sc, SC_PER_BLK)
                slot = sc % 2
                if t == 0:
                    tensor.wait_ge(in0_sems[cc], 16)
                elif cc == 0:
                    # block t landed: its slot sem counts blocks == t (mod 3)
                    # in [1, t], which are (t-1)//3 + 1 DMAs
                    tensor.wait_ge(in_slot_sems[t % 3], 16 * ((t - 1) // 3 + 1))
                if sc >= 2:
                    # PSUM slot reuse: evictions of super-chunk sc-2 done
                    tensor.wait_ge(evA_sem, sc - 1)
                    tensor.wait_ge(evV_sem, sc - 1)
                off = cc * SC_POS
                for s, sem in ((0, mmA_sem), (1, mmV_sem)):
                    last = None
                    for r in (0, 1):
                        for k in (0, 1):
                            for b in (0, 1):
                                last = tensor.matmul(
                                    ps[r][s][slot][b * 64 : (b + 1) * 64, :],
                                    lhsT=wtile(r, s, k),
                                    rhs=rhs(t, b, k, off, SC_POS),
                                    start=(k == 0),
                                    stop=(k == 1),
                                )
                    last.then_inc(sem, 1)

        def ev_dst(sc, r, s):
            t, cc = divmod(sc, SC_PER_BLK)
            rp0 = (t % 2) * 16 + cc * 4
            return obv[:, rp0 : rp0 + 4, r, :, s]

        @block.scalar
        def _(scalar: "bass.BassEngine"):
            for sc in range(NSC):
                t, cc = divmod(sc, SC_PER_BLK)
                slot = sc % 2
                if cc == 0 and t >= 2:
                    # out_buf slot reuse: block t-2's out DMAs done; its slot
                    # sem counts blocks == t (mod 2) in [0, t-2] = t//2 blocks
                    scalar.wait_ge(od_sems[t % 2], 16 * SC_PER_BLK * (t // 2))
                scalar.wait_ge(mmA_sem, sc + 1)
                for r in (0, 1):
                    src = ps[r][0][slot][:].rearrange("p (h w) -> p h w", w=128)
                    ev = scalar.copy(out=ev_dst(sc, r, 0), in_=src)
                ev.then_inc(evA_sem, 1)
                # superchunk evicted on both engines -> issue its output DMA
                # (HWDGE from ACT): 128 partitions x 8 KiB contiguous runs,
                # keeps the drain fine-grained so the tail is ~1 superchunk
                scalar.wait_ge(evV_sem, sc + 1)
                sb = out_buf[
                    :,
                    (t % 2) * SLOT + cc * (SLOT // SC_PER_BLK) : (t % 2) * SLOT
                    + (cc + 1) * (SLOT // SC_PER_BLK),
                ]
                scalar.dma_start(out=yvsc[sc % N_SC], in_=sb).then_inc(
                    od_sems[t % 2], 16
                )

        @block.vector
        def _(vector: "bass.BassEngine"):
            for sc in range(NSC):
                t, cc = divmod(sc, SC_PER_BLK)
                slot = sc % 2
                if cc == 0 and t >= 2:
                    vector.wait_ge(od_sems[t % 2], 16 * SC_PER_BLK * (t // 2))
                vector.wait_ge(mmV_sem, sc + 1)
                for r in (0, 1):
                    src = ps[r][1][slot][:].rearrange("p (h w) -> p h w", w=128)
                    ev = vector.tensor_copy(ev_dst(sc, r, 1), src)
                ev.then_inc(evV_sem, 1)

    return nc


def _get_nc(reps=1):
    key = ("nc", reps)
    if key not in _CACHE:
        _CACHE[key] = _build_nc(reps)
    return _CACHE[key]


def run_on_cores(x, w_all, trace=False):
    from concourse.bass_utils import run_bass_kernel_spmd

    nc = _get_nc()
    x = np.ascontiguousarray(x, dtype=np.float32)
    in_maps = [
        {
            "xs": x[i * B_PER_CORE : (i + 1) * B_PER_CORE],
            "w": w_all,
        }
        for i in range(N_CORES)
    ]
    res = run_bass_kernel_spmd(nc, in_maps, list(range(N_CORES)), trace=trace)
    out = np.concatenate([res.results[i]["ys"] for i in range(N_CORES)], axis=0)
    return out, res


def kernel(x, conv1x1_w, lp_v, hp_v, lp_h, hp_h):
    w_all = _effective_weights(
        np.asarray(conv1x1_w),
        np.asarray(lp_v),
        np.asarray(hp_v),
        np.asarray(lp_h),
        np.asarray(hp_h),
    )
    out, _ = run_on_cores(np.asarray(x), w_all, trace=False)
    return out

